# revision 6
# baseline (speedup 1.0000x reference)
"""Trainium2 Bass kernel for: Conv3d(3->16, k=3, VALID) -> min over depth -> softmax(channels).

Full inputs:  x [8, 3, 24, 128, 128] f32, conv_weight [16, 3, 3, 3, 3] f32
Full output:  [8, 16, 126, 126] f32
Sharding: data-parallel over batch, one sample per NeuronCore (8 cores).

Per-core scheme ("depth-pair split", 2 matmul passes per output point; the
v2 kernel needed 3 passes (one per kd) = 133k PE cycles; this needs ~91k):
  - Output block = (2 out-depths dl, 4 out-rows hl, 16 oc) = full M=128.
    Column dim N = w (126). 32 h-blocks hb (hb 31 is a 2-row tail), 11
    depth-blocks db (out depths 2db, 2db+1).
  - Contraction rows r = (c 3, ddl 2, hh 6, kw 3) = 108 (K padded to 128):
    depth-PAIR tiles T_j hold input depths {2j, 2j+1}; an output block
    accumulates TWO passes in PSUM:
      PSUM[db] = A . T_db  +  B . T_{db+1}
    A[r,m] = W[oc, c, ddl-dl,   hh-hl, kw] (valid kd in {0,1})
    B[r,m] = W[oc, c, ddl+2-dl, hh-hl, kw] (valid kd in {1,2})
    No depth halo in the packed input (each input depth packed once per
    hb); hh halo 6/4, kw replication 3x -> 10.45 MB/sample fp16 vs the
    v2 kernel's 8.7 MB, but PE drops 3 passes -> 2.
  - DMA (measured): big serial transfers with contiguous per-partition
    source on ONE queue sustain ~295-310 B/ns; 32 per-hb DMAs only ~197,
    and splitting across queues is WORSE (concurrent transfers thrash).
    So host packs xp [4, 108, 8, 12, 126] fp16 r-major (h >= 128 rows
    zeroed) and each GROUP of 8 hbs is one 2.61 MB DMA (24.2 KB
    contiguous per partition) into a 3-slot SBUF ring -> input stream
    ~34.5 us on SP, under PE. Output rides ONE 516 KB DMA on the gpsimd
    (Pool) queue at body end. Rows 108:128 of each ring slot are zeroed
    once pre-loop (offset memset fails BIR verify; zero rows x zero lhsT
    rows keep the K=128 contraction exact).
  - Per hb: 6 matmuls (A: j 0:4/4:8/8:11, B: j 1:5/5:9/9:12) into three
    PSUM bank tiles pa[4],pb[4],pc[3] (tags, bufs=2 each) -> only 2
    Ldweights (A,B) per hb since consecutive matmuls share lhsT.
  - min over 22 depths, split across engines so everything fits under PE
    (~1.23 us/hb): Act (~1.0 us/hb) copies pa,pb (8 dbs) PSUM->SBUF fp16
    qc; DVE (~0.95 us/hb) X-min-reduces pc (3 dbs, fp32 1/cyc), folds a
    fp16 2x TT tree over qc, then the dl-fold min(m1[0:64], m1[64:128])
    into the hb-pair mn tile (even hb -> partitions 0:64, odd -> 64:128).
  - softmax over oc per hb-pair g (16 pairs): exp(mn) -> st = ob.et (PE
    broadcast-sum matmul, block-diag-16 ob) -> ln -> mn-ln (DVE) -> exp
    into ot_all[:, g, :]. Steps scheduled 2..6 hbs after the pair
    completes so the in-order Act/PE queues never head-block conv work.
    Act table 6 preloaded (exp+ln+copy in one set).
  - Output yp [128, 16, 126] fp16 ((parity,hl,oc), pair, w); host
    unshuffles/upcasts; hb=31 rows h>=126 are finite garbage, dropped.
"""

import functools
import os
import sys

import numpy as np

os.environ.setdefault("MYCRO_LOCAL_CACHE", "1")
if os.path.isdir("/opt/trn_rl_repo") and "/opt/trn_rl_repo" not in sys.path:
    sys.path.insert(0, "/opt/trn_rl_repo")

import concourse.bacc as bacc
import concourse.mybir as mybir
import concourse.tile as tile
from concourse import bass_utils

C, D, H, W = 3, 24, 128, 128
OC, KD, KH, KW = 16, 3, 3, 3
DO, HO, WO = D - 2, H - 2, W - 2  # 22, 126, 126
NCORES = 8
NHB = 32  # h blocks of 4 out rows; hb 31 -> only hl 0,1 valid
NDB = 11  # depth blocks of 2 out depths
NJ = 12  # depth-pair input tiles
NR = 108  # contraction rows (c, ddl, hh, kw)
NPAIR = 16  # softmax hb-pairs
NG, GH = 4, 8  # input DMA groups of GH h-blocks
NSLOT = 3
F32 = mybir.dt.float32
F16 = mybir.dt.float16
AF = mybir.ActivationFunctionType


def _ridx(c, ddl, hh, kw):
    return ((c * 2 + ddl) * 6 + hh) * 3 + kw


def _pack_weights(w: np.ndarray):
    """lhsT A/B [128, 128] f32 (zero-padded) + ob [128, 128]."""
    A = np.zeros((128, 128), dtype=np.float32)
    B = np.zeros((128, 128), dtype=np.float32)
    for c in range(C):
        for ddl in range(2):
            for hh in range(6):
                for kw in range(KW):
                    r = _ridx(c, ddl, hh, kw)
                    for dl in range(2):
                        for hl in range(4):
                            kh = hh - hl
                            if not (0 <= kh < KH):
                                continue
                            m0 = dl * 64 + hl * 16
                            kdA = ddl - dl
                            if 0 <= kdA < KD:
                                A[r, m0 : m0 + OC] = w[:, c, kdA, kh, kw]
                            kdB = ddl + 2 - dl
                            if 0 <= kdB < KD:
                                B[r, m0 : m0 + OC] = w[:, c, kdB, kh, kw]
    ob = np.zeros((128, 128), dtype=np.float32)
    for pp in range(128):
        g0 = (pp // OC) * OC
        ob[pp, g0 : g0 + OC] = 1.0
    return A, B, ob


def _pack_xp(x1: np.ndarray) -> np.ndarray:
    """x [3,24,128,128] f32 -> xp [NG, NR, GH, NJ, WO] f16 (h>=128 rows zero)."""
    xpad = np.zeros((C, D, H + 2, W), dtype=np.float16)
    xpad[:, :, :H, :] = x1
    xph = np.empty((NR, NHB, NJ, WO), dtype=np.float16)
    for c in range(C):
        for ddl in range(2):
            a = xpad[c, ddl::2]  # [NJ, H+2, W] depth 2j+ddl
            for hh in range(6):
                for kw in range(KW):
                    r = _ridx(c, ddl, hh, kw)
                    # [hb, j, w]: a[j, 4*hb+hh, kw+w]
                    xph[r] = np.transpose(
                        a[:, hh : hh + 4 * NHB : 4, kw : kw + WO], (1, 0, 2)
                    )
    # [r, (g gh), j, w] -> [g, r, gh, j, w]
    return np.ascontiguousarray(
        xph.reshape(NR, NG, GH, NJ, WO).transpose(1, 0, 2, 3, 4)
    )


def build_program(reps: int = 1, stage2: str = "full", unroll: int = 1,
                  convonly: bool = False):
    """reps > 1 wraps the per-sample body in a hardware loop (dev timing only).
    unroll > 1 emits the body N times with no loop (dev: cross-rep pipelining).
    stage2: none | exp | smmm | full; convonly: matmuls+DMAs only (dev)."""
    nc = bacc.Bacc(
        "TRN2",
        target_bir_lowering=False,
        debug=False,
        enable_asserts=True,
        num_devices=NCORES,
    )
    xp_d = nc.dram_tensor("xp", [NG, NR, GH, NJ, WO], F16, kind="ExternalInput").ap()
    xz_d = nc.dram_tensor("xz", [20, GH, NJ, WO], F16, kind="ExternalInput").ap()
    lwA_d = nc.dram_tensor("lwA", [128, 128], F16, kind="ExternalInput").ap()
    lwB_d = nc.dram_tensor("lwB", [128, 128], F16, kind="ExternalInput").ap()
    ob_d = nc.dram_tensor("ob", [128, 128], F16, kind="ExternalInput").ap()
    yp_d = nc.dram_tensor("yp", [128, NPAIR, WO], F16, kind="ExternalOutput").ap()

    with tile.TileContext(nc) as tc:
        with (
            tc.tile_pool(name="const", bufs=1) as cpool,
            tc.tile_pool(name="sm", bufs=3) as spool,
            tc.tile_pool(name="qps", bufs=1, space="PSUM") as qpool,
            tc.tile_pool(name="sps", bufs=2, space="PSUM") as smpool,
        ):
            lwA_sb = cpool.tile([128, 128], F16)
            nc.sync.dma_start(lwA_sb[:], lwA_d)
            lwB_sb = cpool.tile([128, 128], F16)
            nc.sync.dma_start(lwB_sb[:], lwB_d)
            ob_sb = cpool.tile([128, 128], F16)
            nc.sync.dma_start(ob_sb[:], ob_d)

            # Pre-place the combined exp+ln+copy act table (set 6) so the
            # chooser never swaps tables mid-run (1283ns per LoadActFuncSet).
            lset = mybir.InstLoadActFuncSet(
                name=nc.get_next_instruction_name(), act_func_set_id=6
            )
            lset.engine = mybir.EngineType.Activation
            nc.add_instruction(lset)

            # input ring: rows NR:128 of every slot zeroed ONCE via DMA
            xta = cpool.tile([128, NSLOT, GH, NJ, WO], F16)
            for _s in range(NSLOT):
                nc.scalar.dma_start(xta[NR:128, _s, :, :, :], xz_d)

            def emit_body():
                state = {}

                def softmax_step(step, g):
                    if step == 0 and stage2 != "none":
                        et = spool.tile([128, WO], F16, tag="et", bufs=3, name=f"et{g}")
                        nc.scalar.activation(et[:], state[g]["mn"][:], AF.Exp)
                        state[g]["et"] = et
                    if stage2 in ("none", "exp"):
                        return
                    if step == 1:
                        st = smpool.tile([128, WO], F32, tag="ss", name=f"st{g}")
                        nc.tensor.matmul(st[:], ob_sb[:], state[g]["et"][:], start=True, stop=True)
                        state[g]["st"] = st
                    elif step == 2:
                        lt = spool.tile([128, WO], F16, tag="lt", bufs=3, name=f"lt{g}")
                        nc.scalar.activation(lt[:], state[g]["st"][:], AF.Ln)
                        state[g]["lt"] = lt
                    elif step == 3:
                        dt = spool.tile([128, WO], F16, tag="dt", bufs=3, name=f"dt{g}")
                        nc.vector.tensor_tensor(
                            dt[:], state[g]["mn"][:], state[g]["lt"][:],
                            op=mybir.AluOpType.subtract,
                        )
                        state[g]["dt"] = dt
                    elif step == 4:
                        if stage2 == "smmm":
                            return
                        nc.scalar.activation(
                            state["ota"][:, g, :], state[g]["dt"][:], AF.Exp
                        )

                # schedule[hb] = [(step, g)]: pair g's mn completes at hb=2g+1;
                # start its chain 1 hb later, one step per hb.
                schedule = {}
                for g in range(NPAIR):
                    for step, off in enumerate((2, 3, 4, 5, 6)):
                        schedule.setdefault(2 * g + 1 + off, []).append((step, g))

                ota = spool.tile([128, NPAIR, WO], F16, tag="ota", bufs=2, name="ota")
                state["ota"] = ota

                for hb in range(NHB):
                    g = hb // 2
                    if hb % 2 == 0:
                        state[g] = {
                            "mn": spool.tile([128, WO], F16, tag="mn", bufs=4, name=f"mn{g}")
                        }
                    grp, hbl = hb // GH, hb % GH
                    slot = grp % NSLOT
                    if hbl == 0:
                        nc.sync.dma_start(xta[:NR, slot, :, :, :], xp_d[grp].bitcast(F16))

                    pa = qpool.tile([128, 4, WO], F32, tag="pa", bufs=2)
                    pb = qpool.tile([128, 4, WO], F32, tag="pb", bufs=2)
                    pc = qpool.tile([128, 3, WO], F32, tag="pc", bufs=2)
                    # A pass (lhsT=A, depth-pair tiles j=db), then B pass
                    # (lhsT=B, j=db+1): 2 Ldweights per hb total.
                    for lw_sb, j0, start in ((lwA_sb, 0, True), (lwB_sb, 1, False)):
                        for pt, lo, nd in ((pa, 0, 4), (pb, 4, 4), (pc, 8, 3)):
                            nc.tensor.matmul(
                                pt[:],
                                lw_sb[:],
                                xta[:, slot, hbl, j0 + lo : j0 + lo + nd, :],
                                start=start,
                                stop=not start,
                            )

                    if convonly:
                        continue

                    # min over depth: Act downcasts pa/pb; DVE reduces pc
                    # straight from PSUM, folds the fp16 tree + dl-fold.
                    qc = spool.tile([128, 8, WO], F16, tag="qc", bufs=3)
                    nc.scalar.activation(qc[:, 0:4, :], pa[:], AF.Copy)
                    nc.scalar.activation(qc[:, 4:8, :], pb[:], AF.Copy)
                    r2 = spool.tile([128, WO], F16, tag="r2", bufs=3)
                    nc.vector.tensor_reduce(
                        r2[:],
                        pc[:].rearrange("m j w -> m w j"),
                        axis=mybir.AxisListType.X,
                        op=mybir.AluOpType.min,
                    )
                    u = spool.tile([128, 4, WO], F16, tag="u", bufs=3)
                    nc.vector.tensor_tensor(u[:], qc[:, 0:4, :], qc[:, 4:8, :], op=mybir.AluOpType.min)
                    v = spool.tile([128, 2, WO], F16, tag="v", bufs=3)
                    nc.vector.tensor_tensor(v[:], u[:, 0:2, :], u[:, 2:4, :], op=mybir.AluOpType.min)
                    w2 = spool.tile([128, WO], F16, tag="w2", bufs=3)
                    nc.vector.tensor_tensor(w2[:], v[:, 0, :], v[:, 1, :], op=mybir.AluOpType.min)
                    m1 = spool.tile([128, WO], F16, tag="m1", bufs=3)
                    nc.vector.tensor_tensor(m1[:], w2[:], r2[:], op=mybir.AluOpType.min)
                    # dl-fold into the hb-pair mn tile. TensorTensor requires
                    # all SB operands to share a start partition
                    # (birverifier checkSBSameStartPartition), so Act first
                    # copies one dl half across the partition boundary
                    # (partition-offset Activation passes the verifier),
                    # picking the direction so the TT lands in the correct
                    # half of mn: even hb -> mn[0:64], odd -> mn[64:128].
                    mn = state[g]["mn"]
                    m1s = spool.tile([128, WO], F16, tag="m1s", bufs=3)
                    lo = 0 if hb % 2 == 0 else 64
                    hi = 64 - lo
                    nc.scalar.activation(
                        m1s[lo : lo + 64, :], m1[hi : hi + 64, :], AF.Copy
                    )
                    nc.vector.tensor_tensor(
                        mn[lo : lo + 64, :], m1[lo : lo + 64, :],
                        m1s[lo : lo + 64, :], op=mybir.AluOpType.min,
                    )

                    for step, gg in schedule.get(hb, []):
                        softmax_step(step, gg)

                if convonly:
                    return
                for at in sorted(k for k in schedule if k >= NHB):
                    for step, gg in schedule[at]:
                        softmax_step(step, gg)
                if stage2 == "full":
                    # one output DMA on the (otherwise idle) gpsimd queue
                    nc.gpsimd.dma_start(yp_d, state["ota"][:])

            if reps == 1:
                for _ in range(unroll):
                    emit_body()
            else:
                with tc.For_i(0, reps, 1, hint_engines=tuple(mybir.ALL_ENGINES), staggered_reset=True):
                    for _ in range(unroll):
                        emit_body()

    nc.compile()
    return nc


@functools.lru_cache(maxsize=1)
def _program():
    return build_program()


def make_in_maps(x: np.ndarray, w: np.ndarray):
    A, B, ob = _pack_weights(w)
    xz = np.zeros((20, GH, NJ, WO), dtype=np.float16)
    common = {
        "xz": xz,
        "lwA": A.astype(np.float16),
        "lwB": B.astype(np.float16),
        "ob": ob.astype(np.float16),
    }
    return [{"xp": _pack_xp(x[i]), **common} for i in range(x.shape[0])]


def _unpack_yp(yp: np.ndarray) -> np.ndarray:
    """yp [128, NPAIR, WO] f16 -> y [16, 126, 126] f32."""
    v = yp.transpose(1, 0, 2).reshape(NPAIR, 2, 4, OC, WO)  # [g, parity, hl, oc, w]
    y = np.transpose(v, (3, 0, 1, 2, 4)).reshape(OC, NPAIR * 8, WO)
    return y[:, :HO, :].astype(np.float32)


def kernel(x, conv_weight):
    x = np.ascontiguousarray(np.asarray(x, dtype=np.float32))
    w = np.ascontiguousarray(np.asarray(conv_weight, dtype=np.float32))
    assert x.shape == (NCORES, C, D, H, W), x.shape
    nc = _program()
    in_maps = make_in_maps(x, w)
    res = bass_utils.run_bass_kernel_spmd(nc, in_maps, core_ids=list(range(NCORES)))
    out = np.stack([_unpack_yp(res.results[i]["yp"]) for i in range(NCORES)])
    return out
